# revision 5
# baseline (speedup 1.0000x reference)
"""Trainium2 Bass kernel for CAConv2 (coordinate-attention + 3x3 conv block).

Shapes (hardcoded): x (8, 128, 128, 128) f32; data-parallel over batch,
one image per NeuronCore (8 cores).
"""

import numpy as np
import ml_dtypes

import concourse.bacc as bacc
import concourse.tile as tile
from concourse import mybir
from concourse.bass import ds
from concourse.bass_utils import run_bass_kernel_spmd

BF16 = mybir.dt.bfloat16
F32 = mybir.dt.float32
C, H, W, MIP = 128, 128, 128, 8
WP = W + 4  # padded width: cols [2, 130) hold data, 0/1 and 130/131 are zero
HP = H + 2  # padded height: rows [1, 129) hold data
EPS = 1e-5
AF = mybir.ActivationFunctionType
ALU = mybir.AluOpType
AX = mybir.AxisListType

_CACHE = {}


def build_nc():
    nc = bacc.Bacc(num_swdge_queues=2)
    xp = nc.declare_dram_parameter("x", [C, H * W], BF16, isOutput=False)
    w1t = nc.declare_dram_parameter("w1t", [C, MIP], BF16, isOutput=False)
    w1ts = nc.declare_dram_parameter("w1ts", [C, 3 * MIP], BF16, isOutput=False)
    wht = nc.declare_dram_parameter("wht", [MIP, C], BF16, isOutput=False)
    wwt = nc.declare_dram_parameter("wwt", [MIP, C], BF16, isOutput=False)
    # wct[i, k, o] = wc[o, i, k//3, k%3]
    wct = nc.declare_dram_parameter("wct", [C, 9 * C], BF16, isOutput=False)
    # p8 cols: 0: s1/6, 1: t1f/6, 2: s1, 3: t1f+3   (t1f = s1*b1 + be1 - m1*s1)
    p8 = nc.declare_dram_parameter("p8", [MIP, 4], F32, isOutput=False)
    # p128 cols: 0: bh, 1: bw, 2: s2, 3: b2 (= bc*s2 + be2 - m2*s2)
    p128 = nc.declare_dram_parameter("p128", [C, 4], F32, isOutput=False)
    outp = nc.declare_dram_parameter("out", [C, H, W], F32, isOutput=True)

    c1, c2, c3 = 7.0 / 128, 3.0 / 128, 1.0 / 128

    with tile.TileContext(nc) as tc:
        with (
            tc.tile_pool(name="sing", bufs=1) as sing,
            tc.tile_pool(name="pp", bufs=2) as pp,
            tc.tile_pool(name="small", bufs=1) as small,
        ):
            xs = sing.tile([C, H * W], BF16)
            ug = sing.tile([C, HP, WP], BF16)
            s32 = sing.tile([C, H, 4], F32)

            # weights/params ride the sync ring
            w1t_sb = sing.tile([C, MIP], BF16)
            nc.sync.dma_start(out=w1t_sb, in_=w1t[:, :])
            w1ts_sb = sing.tile([C, 3, MIP], BF16)
            nc.sync.dma_start(
                out=w1ts_sb, in_=w1ts.rearrange("c (r m) -> c r m", r=3)
            )
            # x chunks all on the gpsimd SWDGE ring: descriptors drain in
            # issue order -> staggered completion at full BW. Small tail
            # chunks shorten the post-input critical chain.
            XCH = [(0, 16), (16, 16), (32, 32), (64, 32), (96, 16), (112, 8), (120, 8)]
            for r0, nr in XCH:
                nc.gpsimd.dma_start(
                    out=xs[:, ds(r0 * W, nr * W)],
                    in_=xp[:, ds(r0 * W, nr * W)],
                )
            wht_sb = sing.tile([MIP, C], BF16)
            nc.sync.dma_start(out=wht_sb, in_=wht[:, :])
            wwt_sb = sing.tile([MIP, C], BF16)
            nc.sync.dma_start(out=wwt_sb, in_=wwt[:, :])
            p8_sb = sing.tile([MIP, 4], F32)
            nc.sync.dma_start(out=p8_sb, in_=p8[:, :])
            p128_sb = sing.tile([C, 4], F32)
            nc.sync.dma_start(out=p128_sb, in_=p128[:, :])
            wct_sb = sing.tile([C, 9, C], BF16)
            nc.sync.dma_start(out=wct_sb, in_=wct.rearrange("i (k o) -> i k o", k=9))

            # conv padding border of ug (DVE is idle this early)
            nc.vector.memset(ug[:, 0, :], 0.0)
            nc.vector.memset(ug[:, HP - 1, :], 0.0)
            nc.vector.memset(ug[:, 1 : HP - 1, 0:2], 0.0)
            nc.vector.memset(ug[:, 1 : HP - 1, WP - 2 : WP], 0.0)

            # preload ACT function tables off the critical path
            dummy = small.tile([C, 2], F32)
            nc.vector.memset(dummy, 0.0)
            dump = small.tile([C, 2], F32)
            for fn in (AF.Silu, AF.Sigmoid, AF.Copy):
                nc.scalar.activation(dump, dummy, fn, bias=0.0, scale=1.0)

            with (
                tc.tile_pool(name="psA", bufs=1, space="PSUM") as psA,
                tc.tile_pool(name="psB", bufs=4, space="PSUM") as psB,
                tc.tile_pool(name="obp", bufs=4) as obp,
            ):
                # x_w: two-row matmuls with range-prescaled w1 accumulate
                # the weighted row-pool directly onto ONE (8, 2, W) psum tile
                ps_xw = psA.tile([MIP, 2, W], F32, tag="xw")
                ps_yh = psA.tile([MIP, H], F32, tag="yh")
                ps_ah = psA.tile([C, H], F32, tag="ah")
                ah_sb = small.tile([C, H], BF16)

                def emit_tree(r0, nr):
                    # 32-col segment sums for rows [r0, r0+nr): one DVE
                    # reduce per group (vs a 5-op halving tree)
                    xc = xs[:, ds(r0 * W, nr * W)].rearrange(
                        "p (y q s) -> p y q s", q=4, s=32
                    )
                    nc.vector.tensor_reduce(
                        out=s32[:, ds(r0, nr), :], in_=xc, axis=AX.X, op=ALU.add
                    )

                def bn_hswish(src, dst, n):
                    # dst = h_swish(s1*src + t1f) for an (MIP, n) slice
                    z6 = pp.tile([MIP, n], F32, tag="bn_z6")
                    nc.vector.tensor_scalar(
                        out=z6, in0=src, scalar1=p8_sb[:, 0:1],
                        scalar2=p8_sb[:, 1:2], op0=ALU.mult, op1=ALU.add,
                    )
                    _hswish_tail(z6, dst, n)

                def _hswish_tail(z6, dst, n):
                    r = pp.tile([MIP, n], F32, tag="bn_r")
                    nc.vector.tensor_scalar(
                        out=r, in0=z6, scalar1=6.0, scalar2=3.0,
                        op0=ALU.mult, op1=ALU.add,
                    )
                    rc = pp.tile([MIP, n], F32, tag="bn_rc")
                    nc.vector.tensor_scalar(
                        out=rc, in0=r, scalar1=0.0, scalar2=6.0,
                        op0=ALU.max, op1=ALU.min,
                    )
                    nc.vector.tensor_mul(dst, z6, rc)

                def xh_pool(rlo, rhi):
                    # combine s32 rows [rlo, rhi) -> pooled -> yh matmul
                    n = rhi - rlo
                    slh = s32[:, ds(rlo, n), :]
                    tmpA = pp.tile([C, n], F32, tag="tmpA")
                    nc.vector.tensor_add(tmpA, slh[:, :, 2], slh[:, :, 3])
                    m0 = pp.tile([C, n], F32, tag="m0")
                    nc.vector.tensor_scalar_mul(m0, slh[:, :, 0], c1)
                    m1 = pp.tile([C, n], F32, tag="m1")
                    nc.vector.scalar_tensor_tensor(
                        out=m1, in0=slh[:, :, 1], scalar=c2, in1=m0,
                        op0=ALU.mult, op1=ALU.add,
                    )
                    xhp = pp.tile([C, n], BF16, tag="xhp")
                    nc.vector.scalar_tensor_tensor(
                        out=xhp, in0=tmpA, scalar=c3, in1=m1,
                        op0=ALU.mult, op1=ALU.add,
                    )
                    nc.tensor.matmul(
                        ps_yh[:, ds(rlo, n)], w1t_sb, xhp, start=True, stop=True
                    )

                def xh_attn(rlo, rhi):
                    # bn -> h_swish -> ah matmul -> sigmoid for [rlo, rhi)
                    n = rhi - rlo
                    xh_sh = pp.tile([MIP, n], BF16, tag="xh_sh")
                    bn_hswish(ps_yh[:, ds(rlo, n)], xh_sh, n)
                    nc.tensor.matmul(
                        ps_ah[:, ds(rlo, n)], wht_sb, xh_sh, start=True, stop=True
                    )
                    nc.scalar.activation(
                        ah_sb[:, ds(rlo, n)], ps_ah[:, ds(rlo, n)],
                        AF.Sigmoid, bias=p128_sb[:, 0:1], scale=1.0,
                    )

                def gate_rows(rlo, rhi):
                    # ug rows = x * a_h[c,y] * a_w[c,x]; batched multi-row
                    # ops with stride-0 broadcast APs (vs per-row ops, which
                    # are ~330ns each, fixed-latency dominated).
                    R = rhi - rlo
                    xv = xs[:, ds(rlo * W, R * W)].rearrange(
                        "p (r w) -> p r w", r=R
                    )
                    t = pp.tile([C, 16, W], BF16, tag="gt")
                    aw_b = aw_sb[:, :].unsqueeze(1).broadcast_to([C, R, W])
                    nc.vector.tensor_mul(t[:, :R], xv, aw_b)
                    ah_b = (
                        ah_sb[:, ds(rlo, R)].unsqueeze(2).broadcast_to([C, R, W])
                    )
                    nc.vector.tensor_mul(
                        ug[:, 1 + rlo : 1 + rhi, 2 : 2 + W], t[:, :R], ah_b
                    )

                def conv_block(rb):
                    pso = psB.tile([C, 4, W], F32, tag="pso")
                    for k in range(9):
                        dy, dx = k // 3, k % 3
                        nc.tensor.matmul(
                            pso,
                            wct_sb[:, k, :],
                            ug[:, 4 * rb + dy : 4 * rb + dy + 4, 1 + dx : 1 + dx + W],
                            start=(k == 0),
                            stop=(k == 8),
                        )
                    if rb < H // 4 - 1:
                        ob = obp.tile([C, 4, W], F32, tag="ob")
                        nc.scalar.activation(
                            ob, pso, AF.Silu,
                            bias=p128_sb[:, 3:4], scale=p128_sb[:, 2:3],
                        )
                        nc.sync.dma_start(out=outp[:, 4 * rb : 4 * rb + 4, :], in_=ob)
                    else:
                        # split the last block 2+2 so the final ACT+DMA tail
                        # after the last matmul is half as long
                        for h2 in range(2):
                            ob2 = obp.tile([C, 2, W], F32, tag=f"obt{h2}")
                            nc.scalar.activation(
                                ob2, pso[:, 2 * h2 : 2 * h2 + 2, :], AF.Silu,
                                bias=p128_sb[:, 3:4], scale=p128_sb[:, 2:3],
                            )
                            nc.sync.dma_start(
                                out=outp[:, 4 * rb + 2 * h2 : 4 * rb + 2 * h2 + 2, :],
                                in_=ob2,
                            )

                # ---- chunk-chasing: row matmuls + segment reduces ----
                tree_done = 0  # counts 16-row groups
                for r0, nr in XCH:
                    for b in range(0, nr, 2):
                        row = r0 + b
                        nc.tensor.matmul(
                            ps_xw,
                            w1ts_sb[:, min(row // 32, 2), :],
                            xs[:, ds(row * W, 2 * W)],
                            start=(row == 0),
                            stop=(row == 126),
                        )
                    emit_tree(r0, nr)
                    if r0 + nr == 64:
                        xh_pool(0, 64)
                        xh_attn(0, 64)  # block A

                # ---- a_w: the only chain on the critical path ----
                # xwf = ps0*s1/6 on the ACT engine (overlaps the DVE chase);
                # z6n = ps1*s1/6 + xwf = (y - t1f)/6 in one DVE op; the t1f
                # bias folds into the later ops via p8's columns
                xwf = small.tile([MIP, W], F32)
                nc.scalar.activation(
                    xwf, ps_xw[:, 0, :], AF.Copy, bias=0.0, scale=p8_sb[:, 0:1]
                )
                z6n = small.tile([MIP, W], F32)
                nc.vector.scalar_tensor_tensor(
                    out=z6n, in0=ps_xw[:, 1, :], scalar=p8_sb[:, 0:1],
                    in1=xwf, op0=ALU.mult, op1=ALU.add,
                )
                rw = pp.tile([MIP, W], F32, tag="bn_r")
                nc.vector.tensor_scalar(
                    out=rw, in0=z6n, scalar1=6.0, scalar2=p8_sb[:, 3:4],
                    op0=ALU.mult, op1=ALU.add,
                )
                rcw = pp.tile([MIP, W], F32, tag="bn_rc")
                nc.vector.tensor_scalar(
                    out=rcw, in0=rw, scalar1=0.0, scalar2=6.0,
                    op0=ALU.max, op1=ALU.min,
                )
                xw_s = small.tile([MIP, W], BF16)
                nc.vector.scalar_tensor_tensor(
                    out=xw_s, in0=z6n, scalar=p8_sb[:, 1:2],
                    in1=rcw, op0=ALU.add, op1=ALU.mult,
                )
                ps_aw = psA.tile([C, W], F32, tag="aw")
                nc.tensor.matmul(ps_aw, wwt_sb, xw_s, start=True, stop=True)
                aw_sb = small.tile([C, W], BF16)
                nc.scalar.activation(
                    aw_sb, ps_aw, AF.Sigmoid, bias=p128_sb[:, 1:2], scale=1.0
                )

                # ---- gates and conv interleaved: each conv block is
                # emitted right after the gate group that unlocks it, so
                # dependency-range merging never over-waits ----
                gate_rows(0, 8)
                conv_block(0)
                gate_rows(8, 16)
                conv_block(1)
                conv_block(2)
                xh_pool(64, 128)  # block B pooled lanes + yh matmul
                gate_rows(16, 32)
                for rb in range(3, 7):
                    conv_block(rb)
                xh_attn(64, 128)  # block B bn/ah/sigmoid
                gate_rows(32, 48)
                for rb in range(7, 11):
                    conv_block(rb)
                gate_rows(48, 64)
                for rb in range(11, 15):
                    conv_block(rb)
                gate_rows(64, 80)
                for rb in range(15, 19):
                    conv_block(rb)
                gate_rows(80, 96)
                for rb in range(19, 23):
                    conv_block(rb)
                gate_rows(96, 112)
                for rb in range(23, 27):
                    conv_block(rb)
                gate_rows(112, 128)
                for rb in range(27, 32):
                    conv_block(rb)

    nc.compile()
    return nc


def prep_inputs(x, w1, b1, g1, be1, m1, v1, wh, bh, ww, bw, wc, bc, g2, be2, m2, v2):
    """Host-side prep: per-core input maps (weights replicated)."""
    bf = ml_dtypes.bfloat16
    N = x.shape[0]
    s1 = (g1 / np.sqrt(v1 + EPS)).astype(np.float64)
    t1f = s1 * b1 + be1 - m1 * s1
    p8 = np.stack([s1 / 6.0, t1f / 6.0, s1, t1f + 3.0], axis=1).astype(np.float32)
    s2 = (g2 / np.sqrt(v2 + EPS)).astype(np.float64)
    b2 = bc * s2 + be2 - m2 * s2
    p128 = np.stack([bh, bw, s2, b2], axis=1).astype(np.float32)
    cc = np.array([7.0 / 128, 3.0 / 128, 1.0 / 128])
    w1ts = np.stack([w1.T * c for c in cc], axis=1)              # (C, 3, MIP)
    shared = {
        "w1t": np.ascontiguousarray(w1.T).astype(bf),            # (C, MIP)
        "w1ts": np.ascontiguousarray(w1ts.reshape(C, 3 * MIP)).astype(bf),
        "wht": np.ascontiguousarray(wh.T).astype(bf),            # (MIP, C)
        "wwt": np.ascontiguousarray(ww.T).astype(bf),            # (MIP, C)
        "wct": np.ascontiguousarray(
            np.transpose(wc, (1, 2, 3, 0)).reshape(C, 9 * C)
        ).astype(bf),                                            # [i, (ky kx), o]
        "p8": p8,
        "p128": p128,
    }
    in_maps = []
    for n in range(N):
        m = dict(shared)
        m["x"] = np.ascontiguousarray(x[n].reshape(C, H * W)).astype(bf)
        in_maps.append(m)
    return in_maps


def run(inputs, trace=False):
    if "nc" not in _CACHE:
        _CACHE["nc"] = build_nc()
    nc = _CACHE["nc"]
    in_maps = prep_inputs(**inputs)
    res = run_bass_kernel_spmd(nc, in_maps, core_ids=list(range(8)), trace=trace)
    out = np.stack([np.asarray(res.results[i]["out"]) for i in range(8)], axis=0)
    return out.astype(np.float32), res


def kernel(**inputs) -> np.ndarray:
    out, _ = run(inputs, trace=False)
    return out


# revision 10
# speedup vs baseline: 1.0434x; 1.0434x over previous
"""Trainium2 Bass kernel for CAConv2 (coordinate-attention + 3x3 conv block).

Shapes (hardcoded): x (8, 128, 128, 128) f32; data-parallel over batch,
one image per NeuronCore (8 cores).
"""

import numpy as np
import ml_dtypes

import concourse.bacc as bacc
import concourse.tile as tile
from concourse import mybir
from concourse.bass import ds
from concourse.bass_utils import run_bass_kernel_spmd

BF16 = mybir.dt.bfloat16
F32 = mybir.dt.float32
C, H, W, MIP = 128, 128, 128, 8
WP = W + 4  # padded width: cols [2, 130) hold data, 0/1 and 130/131 are zero
HP = H + 2  # padded height: rows [1, 129) hold data
EPS = 1e-5
AF = mybir.ActivationFunctionType
ALU = mybir.AluOpType
AX = mybir.AxisListType

_CACHE = {}


def build_nc():
    nc = bacc.Bacc(num_swdge_queues=2)
    xp = nc.declare_dram_parameter("x", [C, H * W], BF16, isOutput=False)
    w1t = nc.declare_dram_parameter("w1t", [C, MIP], BF16, isOutput=False)
    w1ts = nc.declare_dram_parameter("w1ts", [C, 3 * MIP], BF16, isOutput=False)
    wht = nc.declare_dram_parameter("wht", [MIP, C], BF16, isOutput=False)
    wwt = nc.declare_dram_parameter("wwt", [MIP, C], BF16, isOutput=False)
    # wct[i, k, o] = wc[o, i, k//3, k%3]
    wct = nc.declare_dram_parameter("wct", [C, 9 * C], BF16, isOutput=False)
    # p8 cols: 0: s1/6, 1: t1f/6, 2: s1, 3: t1f+3   (t1f = s1*b1 + be1 - m1*s1)
    p8 = nc.declare_dram_parameter("p8", [MIP, 4], F32, isOutput=False)
    # p128 cols: 0: bh, 1: bw, 2: s2, 3: b2 (= bc*s2 + be2 - m2*s2)
    p128 = nc.declare_dram_parameter("p128", [C, 4], F32, isOutput=False)
    outp = nc.declare_dram_parameter("out", [C, H, W], F32, isOutput=True)

    c1, c2, c3 = 7.0 / 128, 3.0 / 128, 1.0 / 128

    with tile.TileContext(nc) as tc:
        with (
            tc.tile_pool(name="sing", bufs=1) as sing,
            tc.tile_pool(name="pp", bufs=2) as pp,
            tc.tile_pool(name="small", bufs=1) as small,
        ):
            xs = sing.tile([C, H * W], BF16)
            ug = sing.tile([C, HP, WP], BF16)
            s32 = sing.tile([C, H, 4], F32)

            # weights/params ride the sync ring
            w1t_sb = sing.tile([C, MIP], BF16)
            nc.sync.dma_start(out=w1t_sb, in_=w1t[:, :])
            w1ts_sb = sing.tile([C, 3, MIP], BF16)
            nc.sync.dma_start(
                out=w1ts_sb, in_=w1ts.rearrange("c (r m) -> c r m", r=3)
            )
            # x chunks all on the gpsimd SWDGE ring: descriptors drain in
            # issue order -> staggered completion at full BW. Small tail
            # chunks shorten the post-input critical chain.
            XCH = [(0, 16), (16, 16), (32, 32), (64, 32), (96, 16), (112, 8), (120, 8)]
            for r0, nr in XCH:
                nc.gpsimd.dma_start(
                    out=xs[:, ds(r0 * W, nr * W)],
                    in_=xp[:, ds(r0 * W, nr * W)],
                )
            wht_sb = sing.tile([MIP, C], BF16)
            nc.sync.dma_start(out=wht_sb, in_=wht[:, :])
            wwt_sb = sing.tile([MIP, C], BF16)
            nc.sync.dma_start(out=wwt_sb, in_=wwt[:, :])
            p8_sb = sing.tile([MIP, 4], F32)
            nc.sync.dma_start(out=p8_sb, in_=p8[:, :])
            p128_sb = sing.tile([C, 4], F32)
            nc.sync.dma_start(out=p128_sb, in_=p128[:, :])
            wct_sb = sing.tile([C, 9, C], BF16)
            nc.sync.dma_start(out=wct_sb, in_=wct.rearrange("i (k o) -> i k o", k=9))

            # conv padding border of ug (DVE is idle this early)
            nc.vector.memset(ug[:, 0, :], 0.0)
            nc.vector.memset(ug[:, HP - 1, :], 0.0)
            nc.vector.memset(ug[:, 1 : HP - 1, 0:2], 0.0)
            nc.vector.memset(ug[:, 1 : HP - 1, WP - 2 : WP], 0.0)

            # preload ACT function tables off the critical path
            dummy = small.tile([C, 2], F32)
            nc.vector.memset(dummy, 0.0)
            dump = small.tile([C, 2], F32)
            for fn in (AF.Silu, AF.Sigmoid, AF.Copy):
                nc.scalar.activation(dump, dummy, fn, bias=0.0, scale=1.0)

            with (
                tc.tile_pool(name="psA", bufs=1, space="PSUM") as psA,
                tc.tile_pool(name="psB", bufs=4, space="PSUM") as psB,
                tc.tile_pool(name="obp", bufs=4) as obp,
            ):
                # x_w: two-row matmuls with range-prescaled w1 accumulate
                # the weighted row-pool directly onto ONE (8, 2, W) psum tile
                ps_xw = psA.tile([MIP, 2, W], F32, tag="xw")
                ps_yh = psA.tile([MIP, H], F32, tag="yh")
                ps_ah = psA.tile([C, H], F32, tag="ah")
                ah_sb = small.tile([C, H], BF16)

                def emit_tree(r0, nr):
                    # 32-col segment sums for rows [r0, r0+nr); 5-op halving
                    # tree (measured faster than a single TENSOR_REDUCE, and
                    # finer FIFO quanta chase the DMA better)
                    eng = nc.vector
                    xc = xs[:, ds(r0 * W, nr * W)].rearrange(
                        "p (y q s) -> p y q s", q=4, s=32
                    )
                    t1 = pp.tile([C, 16, 4, 16], BF16, tag="t1")
                    eng.tensor_add(
                        t1[:, :nr], xc[:, :, :, 0:16], xc[:, :, :, 16:32]
                    )
                    t2 = pp.tile([C, 16, 4, 8], BF16, tag="t2")
                    eng.tensor_add(t2[:, :nr], t1[:, :nr, :, 0:8], t1[:, :nr, :, 8:16])
                    t3 = pp.tile([C, 16, 4, 4], BF16, tag="t3")
                    eng.tensor_add(t3[:, :nr], t2[:, :nr, :, 0:4], t2[:, :nr, :, 4:8])
                    t4 = pp.tile([C, 16, 4, 2], BF16, tag="t4")
                    eng.tensor_add(t4[:, :nr], t3[:, :nr, :, 0:2], t3[:, :nr, :, 2:4])
                    sl = s32[:, ds(r0, nr), :]
                    eng.tensor_add(sl, t4[:, :nr, :, 0], t4[:, :nr, :, 1])

                def bn_hswish(src, dst, n):
                    # dst = h_swish(s1*src + t1f) for an (MIP, n) slice
                    z6 = pp.tile([MIP, n], F32, tag="bn_z6")
                    nc.vector.tensor_scalar(
                        out=z6, in0=src, scalar1=p8_sb[:, 0:1],
                        scalar2=p8_sb[:, 1:2], op0=ALU.mult, op1=ALU.add,
                    )
                    _hswish_tail(z6, dst, n)

                def _hswish_tail(z6, dst, n):
                    r = pp.tile([MIP, n], F32, tag="bn_r")
                    nc.vector.tensor_scalar(
                        out=r, in0=z6, scalar1=6.0, scalar2=3.0,
                        op0=ALU.mult, op1=ALU.add,
                    )
                    rc = pp.tile([MIP, n], F32, tag="bn_rc")
                    nc.vector.tensor_scalar(
                        out=rc, in0=r, scalar1=0.0, scalar2=6.0,
                        op0=ALU.max, op1=ALU.min,
                    )
                    nc.vector.tensor_mul(dst, z6, rc)

                def xh_pool(rlo, rhi):
                    # combine s32 rows [rlo, rhi) -> pooled -> yh matmul
                    n = rhi - rlo
                    slh = s32[:, ds(rlo, n), :]
                    tmpA = pp.tile([C, n], F32, tag="tmpA")
                    nc.vector.tensor_add(tmpA, slh[:, :, 2], slh[:, :, 3])
                    m0 = pp.tile([C, n], F32, tag="m0")
                    nc.vector.tensor_scalar_mul(m0, slh[:, :, 0], c1)
                    m1 = pp.tile([C, n], F32, tag="m1")
                    nc.vector.scalar_tensor_tensor(
                        out=m1, in0=slh[:, :, 1], scalar=c2, in1=m0,
                        op0=ALU.mult, op1=ALU.add,
                    )
                    xhp = pp.tile([C, n], BF16, tag="xhp")
                    nc.vector.scalar_tensor_tensor(
                        out=xhp, in0=tmpA, scalar=c3, in1=m1,
                        op0=ALU.mult, op1=ALU.add,
                    )
                    nc.tensor.matmul(
                        ps_yh[:, ds(rlo, n)], w1t_sb, xhp, start=True, stop=True
                    )

                def xh_attn(rlo, rhi):
                    # bn -> h_swish -> ah matmul -> sigmoid for [rlo, rhi)
                    n = rhi - rlo
                    xh_sh = pp.tile([MIP, n], BF16, tag="xh_sh")
                    bn_hswish(ps_yh[:, ds(rlo, n)], xh_sh, n)
                    nc.tensor.matmul(
                        ps_ah[:, ds(rlo, n)], wht_sb, xh_sh, start=True, stop=True
                    )
                    nc.scalar.activation(
                        ah_sb[:, ds(rlo, n)], ps_ah[:, ds(rlo, n)],
                        AF.Sigmoid, bias=p128_sb[:, 0:1], scale=1.0,
                    )

                def gate_rows(rlo, rhi):
                    # ug rows = x * a_h[c,y] * a_w[c,x]; batched multi-row
                    # ops with stride-0 broadcast APs (vs per-row ops, which
                    # are ~330ns each, fixed-latency dominated).
                    R = rhi - rlo
                    xv = xs[:, ds(rlo * W, R * W)].rearrange(
                        "p (r w) -> p r w", r=R
                    )
                    t = pp.tile([C, 16, W], BF16, tag="gt")
                    aw_b = aw_sb[:, :].unsqueeze(1).broadcast_to([C, R, W])
                    nc.vector.tensor_mul(t[:, :R], xv, aw_b)
                    ah_b = (
                        ah_sb[:, ds(rlo, R)].unsqueeze(2).broadcast_to([C, R, W])
                    )
                    nc.vector.tensor_mul(
                        ug[:, 1 + rlo : 1 + rhi, 2 : 2 + W], t[:, :R], ah_b
                    )

                def conv_block(rb):
                    pso = psB.tile([C, 4, W], F32, tag="pso")
                    for k in range(9):
                        dy, dx = k // 3, k % 3
                        nc.tensor.matmul(
                            pso,
                            wct_sb[:, k, :],
                            ug[:, 4 * rb + dy : 4 * rb + dy + 4, 1 + dx : 1 + dx + W],
                            start=(k == 0),
                            stop=(k == 8),
                        )
                    if rb < H // 4 - 1:
                        ob = obp.tile([C, 4, W], F32, tag="ob")
                        nc.scalar.activation(
                            ob, pso, AF.Silu,
                            bias=p128_sb[:, 3:4], scale=p128_sb[:, 2:3],
                        )
                        nc.sync.dma_start(out=outp[:, 4 * rb : 4 * rb + 4, :], in_=ob)
                    else:
                        # split the last block 2+2 so the final ACT+DMA tail
                        # after the last matmul is half as long
                        for h2 in range(2):
                            ob2 = obp.tile([C, 2, W], F32, tag=f"obt{h2}")
                            nc.scalar.activation(
                                ob2, pso[:, 2 * h2 : 2 * h2 + 2, :], AF.Silu,
                                bias=p128_sb[:, 3:4], scale=p128_sb[:, 2:3],
                            )
                            nc.sync.dma_start(
                                out=outp[:, 4 * rb + 2 * h2 : 4 * rb + 2 * h2 + 2, :],
                                in_=ob2,
                            )

                # ---- chunk-chasing: row matmuls + segment trees ----
                # trees for the last two (8-row) chunks are deferred past
                # the a_w chain so the DVE head is free at input end
                for r0, nr in XCH:
                    for b in range(0, nr, 2):
                        row = r0 + b
                        nc.tensor.matmul(
                            ps_xw,
                            w1ts_sb[:, min(row // 32, 2), :],
                            xs[:, ds(row * W, 2 * W)],
                            start=(row == 0),
                            stop=(row == 126),
                        )
                    if r0 + nr <= 112:
                        for t0 in range(r0, r0 + nr, 16):
                            emit_tree(t0, min(16, r0 + nr - t0))
                    if r0 + nr == 64:
                        xh_pool(0, 64)
                        xh_attn(0, 64)  # block A

                # ---- a_w: the only chain on the critical path ----
                # xwf = ps0*s1/6 on the ACT engine (overlaps the DVE chase);
                # z6n = ps1*s1/6 + xwf = (y - t1f)/6 in one DVE op; the t1f
                # bias folds into the later ops via p8's columns
                xwf = small.tile([MIP, W], F32)
                nc.scalar.activation(
                    xwf, ps_xw[:, 0, :], AF.Copy, bias=0.0, scale=p8_sb[:, 0:1]
                )
                z6n = small.tile([MIP, W], F32)
                nc.vector.scalar_tensor_tensor(
                    out=z6n, in0=ps_xw[:, 1, :], scalar=p8_sb[:, 0:1],
                    in1=xwf, op0=ALU.mult, op1=ALU.add,
                )
                rw = pp.tile([MIP, W], F32, tag="bn_r")
                nc.vector.tensor_scalar(
                    out=rw, in0=z6n, scalar1=6.0, scalar2=p8_sb[:, 3:4],
                    op0=ALU.mult, op1=ALU.add,
                )
                rcw = pp.tile([MIP, W], F32, tag="bn_rc")
                nc.vector.tensor_scalar(
                    out=rcw, in0=rw, scalar1=0.0, scalar2=6.0,
                    op0=ALU.max, op1=ALU.min,
                )
                xw_s = small.tile([MIP, W], BF16)
                nc.vector.scalar_tensor_tensor(
                    out=xw_s, in0=z6n, scalar=p8_sb[:, 1:2],
                    in1=rcw, op0=ALU.add, op1=ALU.mult,
                )

                # bridge matmuls: keep the PE's activity monitor busy across
                # the a_w chain so the conv starts at full clock (an idle
                # window >~3.4us re-throttles the PE to half rate). They
                # overwrite ps_yh rows 0..64, already consumed by block A.
                for _ in range(16):
                    nc.tensor.matmul(
                        ps_yh[:, 0:64], w1ts_sb[:, 0, :], xs[:, 0:64],
                        start=True, stop=True,
                    )

                ps_aw = psA.tile([C, W], F32, tag="aw")
                nc.tensor.matmul(ps_aw, wwt_sb, xw_s, start=True, stop=True)
                aw_sb = small.tile([C, W], BF16)
                nc.scalar.activation(
                    aw_sb, ps_aw, AF.Sigmoid, bias=p128_sb[:, 1:2], scale=1.0
                )

                # ---- gates and conv interleaved: each conv block is
                # emitted right after the gate group that unlocks it, so
                # dependency-range merging never over-waits; the deferred
                # trees and block B slot into the DVE stream where their
                # inputs are ready and their consumers are not yet due ----
                gate_rows(0, 8)
                conv_block(0)
                gate_rows(8, 16)
                conv_block(1)
                conv_block(2)
                emit_tree(112, 8)
                emit_tree(120, 8)
                gate_rows(16, 32)
                for rb in range(3, 7):
                    conv_block(rb)
                xh_pool(64, 128)  # block B pooled lanes + yh matmul
                gate_rows(32, 48)
                for rb in range(7, 11):
                    conv_block(rb)
                xh_attn(64, 128)  # block B bn/ah/sigmoid
                gate_rows(48, 64)
                for rb in range(11, 15):
                    conv_block(rb)
                gate_rows(64, 80)
                for rb in range(15, 19):
                    conv_block(rb)
                gate_rows(80, 96)
                for rb in range(19, 23):
                    conv_block(rb)
                gate_rows(96, 112)
                for rb in range(23, 27):
                    conv_block(rb)
                gate_rows(112, 128)
                for rb in range(27, 32):
                    conv_block(rb)

    nc.compile()
    return nc


def prep_inputs(x, w1, b1, g1, be1, m1, v1, wh, bh, ww, bw, wc, bc, g2, be2, m2, v2):
    """Host-side prep: per-core input maps (weights replicated)."""
    bf = ml_dtypes.bfloat16
    N = x.shape[0]
    s1 = (g1 / np.sqrt(v1 + EPS)).astype(np.float64)
    t1f = s1 * b1 + be1 - m1 * s1
    p8 = np.stack([s1 / 6.0, t1f / 6.0, s1, t1f + 3.0], axis=1).astype(np.float32)
    s2 = (g2 / np.sqrt(v2 + EPS)).astype(np.float64)
    b2 = bc * s2 + be2 - m2 * s2
    p128 = np.stack([bh, bw, s2, b2], axis=1).astype(np.float32)
    cc = np.array([7.0 / 128, 3.0 / 128, 1.0 / 128])
    w1ts = np.stack([w1.T * c for c in cc], axis=1)              # (C, 3, MIP)
    shared = {
        "w1t": np.ascontiguousarray(w1.T).astype(bf),            # (C, MIP)
        "w1ts": np.ascontiguousarray(w1ts.reshape(C, 3 * MIP)).astype(bf),
        "wht": np.ascontiguousarray(wh.T).astype(bf),            # (MIP, C)
        "wwt": np.ascontiguousarray(ww.T).astype(bf),            # (MIP, C)
        "wct": np.ascontiguousarray(
            np.transpose(wc, (1, 2, 3, 0)).reshape(C, 9 * C)
        ).astype(bf),                                            # [i, (ky kx), o]
        "p8": p8,
        "p128": p128,
    }
    in_maps = []
    for n in range(N):
        m = dict(shared)
        m["x"] = np.ascontiguousarray(x[n].reshape(C, H * W)).astype(bf)
        in_maps.append(m)
    return in_maps


def run(inputs, trace=False):
    if "nc" not in _CACHE:
        _CACHE["nc"] = build_nc()
    nc = _CACHE["nc"]
    in_maps = prep_inputs(**inputs)
    res = run_bass_kernel_spmd(nc, in_maps, core_ids=list(range(8)), trace=trace)
    out = np.stack([np.asarray(res.results[i]["out"]) for i in range(8)], axis=0)
    return out.astype(np.float32), res


def kernel(**inputs) -> np.ndarray:
    out, _ = run(inputs, trace=False)
    return out


# revision 24
# speedup vs baseline: 1.0823x; 1.0373x over previous
"""Trainium2 Bass kernel for CAConv2 (coordinate-attention + 3x3 conv block).

Shapes (hardcoded): x (8, 128, 128, 128) f32; data-parallel over batch,
one image per NeuronCore (8 cores).
"""

import numpy as np
import ml_dtypes

import concourse.bacc as bacc
import concourse.tile as tile
from concourse import mybir
from concourse.bass import ds
from concourse.bass_utils import run_bass_kernel_spmd

BF16 = mybir.dt.bfloat16
F32 = mybir.dt.float32
C, H, W, MIP = 128, 128, 128, 8
WP = W + 4  # padded width: cols [2, 130) hold data, 0/1 and 130/131 are zero
HP = H + 2  # padded height: rows [1, 129) hold data
EPS = 1e-5
AF = mybir.ActivationFunctionType
ALU = mybir.AluOpType
AX = mybir.AxisListType

_CACHE = {}


def build_nc():
    nc = bacc.Bacc(num_swdge_queues=2)
    xp = nc.declare_dram_parameter("x", [C, H * W], BF16, isOutput=False)
    w1t = nc.declare_dram_parameter("w1t", [C, MIP], BF16, isOutput=False)
    w1ts = nc.declare_dram_parameter("w1ts", [C, 3 * MIP], BF16, isOutput=False)
    wht = nc.declare_dram_parameter("wht", [MIP, C], BF16, isOutput=False)
    wwt = nc.declare_dram_parameter("wwt", [MIP, C], BF16, isOutput=False)
    # wct[i, k, o] = wc[o, i, k//3, k%3]
    wct = nc.declare_dram_parameter("wct", [C, 9 * C], BF16, isOutput=False)
    # p8 cols: 0: s1/6, 1: t1f/6, 2: s1, 3: t1f+3   (t1f = s1*b1 + be1 - m1*s1)
    p8 = nc.declare_dram_parameter("p8", [MIP, 4], F32, isOutput=False)
    # p128 cols: 0: bh, 1: bw, 2: s2, 3: b2 (= bc*s2 + be2 - m2*s2)
    p128 = nc.declare_dram_parameter("p128", [C, 4], F32, isOutput=False)
    outp = nc.declare_dram_parameter("out", [C, H, W], F32, isOutput=True)

    c1, c2, c3 = 7.0 / 128, 3.0 / 128, 1.0 / 128

    with tile.TileContext(nc) as tc:
        with (
            tc.tile_pool(name="sing", bufs=1) as sing,
            tc.tile_pool(name="pp", bufs=2) as pp,
            tc.tile_pool(name="small", bufs=1) as small,
        ):
            xs = sing.tile([C, H * W], BF16)
            ug = sing.tile([C, HP, WP], BF16)
            s32 = sing.tile([C, H, 4], F32)

            # weights/params ride the sync ring
            w1t_sb = sing.tile([C, MIP], BF16)
            nc.sync.dma_start(out=w1t_sb, in_=w1t[:, :])
            w1ts_sb = sing.tile([C, 3, MIP], BF16)
            nc.sync.dma_start(
                out=w1ts_sb, in_=w1ts.rearrange("c (r m) -> c r m", r=3)
            )
            # x chunks all on the gpsimd SWDGE ring: descriptors drain in
            # issue order -> staggered completion at full BW. Small tail
            # chunks shorten the post-input critical chain.
            XCH = [(0, 16), (16, 16), (32, 32), (64, 32), (96, 16), (112, 8), (120, 8)]
            for r0, nr in XCH:
                nc.gpsimd.dma_start(
                    out=xs[:, ds(r0 * W, nr * W)],
                    in_=xp[:, ds(r0 * W, nr * W)],
                )
            wht_sb = sing.tile([MIP, C], BF16)
            nc.sync.dma_start(out=wht_sb, in_=wht[:, :])
            wwt_sb = sing.tile([MIP, C], BF16)
            nc.sync.dma_start(out=wwt_sb, in_=wwt[:, :])
            p8_sb = sing.tile([MIP, 4], F32)
            nc.sync.dma_start(out=p8_sb, in_=p8[:, :])
            p128_sb = sing.tile([C, 4], F32)
            nc.sync.dma_start(out=p128_sb, in_=p128[:, :])
            wct_sb = sing.tile([C, 9, C], BF16)
            nc.sync.dma_start(out=wct_sb, in_=wct.rearrange("i (k o) -> i k o", k=9))

            # conv padding border of ug / ug0 (DVE is idle this early)
            nc.vector.memset(ug[:, 0, :], 0.0)
            nc.vector.memset(ug[:, HP - 1, :], 0.0)
            nc.vector.memset(ug[:, 1 : HP - 1, 0:2], 0.0)
            nc.vector.memset(ug[:, 1 : HP - 1, WP - 2 : WP], 0.0)

            # preload ACT function tables off the critical path
            dummy = small.tile([C, 2], F32)
            nc.vector.memset(dummy, 0.0)
            dump = small.tile([C, 2], F32)
            for fn in (AF.Silu, AF.Sigmoid, AF.Copy):
                nc.scalar.activation(dump, dummy, fn, bias=0.0, scale=1.0)

            with (
                tc.tile_pool(name="psA", bufs=1, space="PSUM") as psA,
                tc.tile_pool(name="psB", bufs=4, space="PSUM") as psB,
                tc.tile_pool(name="obp", bufs=4) as obp,
            ):
                # x_w: two-row matmuls with range-prescaled w1 accumulate
                # the weighted row-pool directly onto ONE (8, 2, W) psum tile
                ps_xw = psA.tile([MIP, 2, W], F32, tag="xw")
                ps_yh = psA.tile([MIP, H], F32, tag="yh")
                ps_ah = psA.tile([C, H], F32, tag="ah")
                ah_sb = small.tile([C, H], BF16)

                def emit_tree(r0, nr):
                    # 32-col segment sums for rows [r0, r0+nr); 5-op halving
                    # tree (measured faster than a single TENSOR_REDUCE, and
                    # finer FIFO quanta chase the DMA better)
                    eng = nc.vector
                    xc = xs[:, ds(r0 * W, nr * W)].rearrange(
                        "p (y q s) -> p y q s", q=4, s=32
                    )
                    t1 = pp.tile([C, 16, 4, 16], BF16, tag="t1")
                    eng.tensor_add(
                        t1[:, :nr], xc[:, :, :, 0:16], xc[:, :, :, 16:32]
                    )
                    t2 = pp.tile([C, 16, 4, 8], BF16, tag="t2")
                    eng.tensor_add(t2[:, :nr], t1[:, :nr, :, 0:8], t1[:, :nr, :, 8:16])
                    t3 = pp.tile([C, 16, 4, 4], BF16, tag="t3")
                    eng.tensor_add(t3[:, :nr], t2[:, :nr, :, 0:4], t2[:, :nr, :, 4:8])
                    t4 = pp.tile([C, 16, 4, 2], BF16, tag="t4")
                    eng.tensor_add(t4[:, :nr], t3[:, :nr, :, 0:2], t3[:, :nr, :, 2:4])
                    sl = s32[:, ds(r0, nr), :]
                    eng.tensor_add(sl, t4[:, :nr, :, 0], t4[:, :nr, :, 1])

                def bn_hswish(src, dst, n):
                    # dst = h_swish(s1*src + t1f) for an (MIP, n) slice
                    z6 = pp.tile([MIP, n], F32, tag="bn_z6")
                    nc.vector.tensor_scalar(
                        out=z6, in0=src, scalar1=p8_sb[:, 0:1],
                        scalar2=p8_sb[:, 1:2], op0=ALU.mult, op1=ALU.add,
                    )
                    _hswish_tail(z6, dst, n)

                def _hswish_tail(z6, dst, n):
                    r = pp.tile([MIP, n], F32, tag="bn_r")
                    nc.vector.tensor_scalar(
                        out=r, in0=z6, scalar1=6.0, scalar2=3.0,
                        op0=ALU.mult, op1=ALU.add,
                    )
                    rc = pp.tile([MIP, n], F32, tag="bn_rc")
                    nc.vector.tensor_scalar(
                        out=rc, in0=r, scalar1=0.0, scalar2=6.0,
                        op0=ALU.max, op1=ALU.min,
                    )
                    nc.vector.tensor_mul(dst, z6, rc)

                def xh_pool(rlo, rhi):
                    # combine s32 rows [rlo, rhi) -> pooled -> yh matmul
                    n = rhi - rlo
                    slh = s32[:, ds(rlo, n), :]
                    tmpA = pp.tile([C, n], F32, tag="tmpA")
                    nc.vector.tensor_add(tmpA, slh[:, :, 2], slh[:, :, 3])
                    m0 = pp.tile([C, n], F32, tag="m0")
                    nc.vector.tensor_scalar_mul(m0, slh[:, :, 0], c1)
                    m1 = pp.tile([C, n], F32, tag="m1")
                    nc.vector.scalar_tensor_tensor(
                        out=m1, in0=slh[:, :, 1], scalar=c2, in1=m0,
                        op0=ALU.mult, op1=ALU.add,
                    )
                    xhp = pp.tile([C, n], BF16, tag="xhp")
                    nc.vector.scalar_tensor_tensor(
                        out=xhp, in0=tmpA, scalar=c3, in1=m1,
                        op0=ALU.mult, op1=ALU.add,
                    )
                    nc.tensor.matmul(
                        ps_yh[:, ds(rlo, n)], w1t_sb, xhp, start=True, stop=True
                    )

                def xh_attn(rlo, rhi):
                    # bn -> h_swish -> ah matmul -> sigmoid for [rlo, rhi)
                    n = rhi - rlo
                    xh_sh = pp.tile([MIP, n], BF16, tag="xh_sh")
                    bn_hswish(ps_yh[:, ds(rlo, n)], xh_sh, n)
                    nc.tensor.matmul(
                        ps_ah[:, ds(rlo, n)], wht_sb, xh_sh, start=True, stop=True
                    )
                    nc.scalar.activation(
                        ah_sb[:, ds(rlo, n)], ps_ah[:, ds(rlo, n)],
                        AF.Sigmoid, bias=p128_sb[:, 0:1], scale=1.0,
                    )

                def gate_rows(rlo, rhi):
                    # ug rows = x * a_h[c,y] * a_w[c,x]; batched multi-row
                    # ops with stride-0 broadcast APs (vs per-row ops, which
                    # are ~330ns each, fixed-latency dominated).
                    R = rhi - rlo
                    xv = xs[:, ds(rlo * W, R * W)].rearrange(
                        "p (r w) -> p r w", r=R
                    )
                    t = pp.tile([C, 16, W], BF16, tag="gt")
                    aw_b = aw_sb[:, :].unsqueeze(1).broadcast_to([C, R, W])
                    nc.vector.tensor_mul(t[:, :R], xv, aw_b)
                    ah_b = (
                        ah_sb[:, ds(rlo, R)].unsqueeze(2).broadcast_to([C, R, W])
                    )
                    nc.vector.tensor_mul(
                        ug[:, 1 + rlo : 1 + rhi, 2 : 2 + W], t[:, :R], ah_b
                    )

                def conv_block(rb):
                    pso = psB.tile([C, 4, W], F32, tag="pso")
                    for k in range(9):
                        dy, dx = k // 3, k % 3
                        nc.tensor.matmul(
                            pso,
                            wct_sb[:, k, :],
                            ug[:, 4 * rb + dy : 4 * rb + dy + 4, 1 + dx : 1 + dx + W],
                            start=(k == 0),
                            stop=(k == 8),
                        )
                    if rb < H // 4 - 1:
                        ob = obp.tile([C, 4, W], F32, tag="ob")
                        nc.scalar.activation(
                            ob, pso, AF.Silu,
                            bias=p128_sb[:, 3:4], scale=p128_sb[:, 2:3],
                        )
                        nc.sync.dma_start(out=outp[:, 4 * rb : 4 * rb + 4, :], in_=ob)
                    else:
                        # split the last block 2+2 so the final ACT+DMA tail
                        # after the last matmul is half as long
                        for h2 in range(2):
                            ob2 = obp.tile([C, 2, W], F32, tag=f"obt{h2}")
                            nc.scalar.activation(
                                ob2, pso[:, 2 * h2 : 2 * h2 + 2, :], AF.Silu,
                                bias=p128_sb[:, 3:4], scale=p128_sb[:, 2:3],
                            )
                            nc.sync.dma_start(
                                out=outp[:, 4 * rb + 2 * h2 : 4 * rb + 2 * h2 + 2, :],
                                in_=ob2,
                            )

                # ---- chunk-chasing: row matmuls + segment trees ----
                # trees for the last two (8-row) chunks are deferred past
                # the a_w chain so the DVE head is free at input end
                for r0, nr in XCH:
                    for b in range(0, nr, 2):
                        row = r0 + b
                        nc.tensor.matmul(
                            ps_xw,
                            w1ts_sb[:, min(row // 32, 2), :],
                            xs[:, ds(row * W, 2 * W)],
                            start=(row == 0),
                            stop=(row == 126),
                        )
                    if r0 + nr <= 96:
                        for t0 in range(r0, r0 + nr, 16):
                            emit_tree(t0, min(16, r0 + nr - t0))
                    if r0 + nr == 64:
                        xh_pool(0, 64)
                        xh_attn(0, 64)  # block A

                # ---- a_w: the only chain on the critical path ----
                # xwf = ps0*s1/6 on the ACT engine (overlaps the DVE chase);
                # z6n = ps1*s1/6 + xwf = (y - t1f)/6 in one DVE op; the t1f
                # bias folds into the later ops via p8's columns
                xwf = small.tile([MIP, W], F32)
                nc.scalar.activation(
                    xwf, ps_xw[:, 0, :], AF.Copy, bias=0.0, scale=p8_sb[:, 0:1]
                )
                z6n = small.tile([MIP, W], F32)
                nc.vector.scalar_tensor_tensor(
                    out=z6n, in0=ps_xw[:, 1, :], scalar=p8_sb[:, 0:1],
                    in1=xwf, op0=ALU.mult, op1=ALU.add,
                )
                rw = pp.tile([MIP, W], F32, tag="bn_r")
                nc.vector.tensor_scalar(
                    out=rw, in0=z6n, scalar1=6.0, scalar2=p8_sb[:, 3:4],
                    op0=ALU.mult, op1=ALU.add,
                )
                rcw = pp.tile([MIP, W], F32, tag="bn_rc")
                nc.vector.tensor_scalar(
                    out=rcw, in0=rw, scalar1=0.0, scalar2=6.0,
                    op0=ALU.max, op1=ALU.min,
                )
                xw_s = small.tile([MIP, W], BF16)
                nc.vector.scalar_tensor_tensor(
                    out=xw_s, in0=z6n, scalar=p8_sb[:, 1:2],
                    in1=rcw, op0=ALU.add, op1=ALU.mult,
                )

                # bridge matmuls: keep the PE's activity monitor busy across
                # the a_w chain so the conv starts at full clock (a sparse
                # ~4us window re-throttles the PE to half rate). Two batches
                # straddle the a_w matmul; they overwrite ps_yh rows 0..64,
                # already consumed by block A.
                for _ in range(10):
                    nc.tensor.matmul(
                        ps_yh[:, 0:128], w1ts_sb[:, 0, :], xs[:, 0:128],
                        start=True, stop=True,
                    )

                ps_aw = psA.tile([C, W], F32, tag="aw")
                nc.tensor.matmul(ps_aw, wwt_sb, xw_s, start=True, stop=True)
                aw_sb = small.tile([C, W], BF16)
                nc.scalar.activation(
                    aw_sb, ps_aw, AF.Sigmoid, bias=p128_sb[:, 1:2], scale=1.0
                )
                # second bridge batch: spans the gate-computation window
                # (~2.5us) so the PE never goes sparse before the conv; the
                # WAR on ps_xw (read by xwf/z6n) times their start
                for _ in range(22):
                    nc.tensor.matmul(
                        ps_xw, w1ts_sb[:, 0, :], xs[:, ds(0, 2 * W)],
                        start=True, stop=True,
                    )

                # ---- gates and conv interleaved: each conv block is
                # emitted right after the gate group that unlocks it; the
                # deferred trees and block B slot into the DVE stream where
                # their inputs are ready and consumers are not yet due ----
                gate_rows(0, 8)
                conv_block(0)
                gate_rows(8, 16)
                conv_block(1)
                conv_block(2)
                gate_rows(16, 32)
                for rb in range(3, 7):
                    conv_block(rb)
                emit_tree(96, 16)
                emit_tree(112, 8)
                emit_tree(120, 8)
                xh_pool(64, 128)  # block B pooled lanes + yh matmul
                gate_rows(32, 48)
                for rb in range(7, 11):
                    conv_block(rb)
                xh_attn(64, 128)  # block B bn/ah/sigmoid
                gate_rows(48, 64)
                for rb in range(11, 15):
                    conv_block(rb)
                gate_rows(64, 80)
                for rb in range(15, 19):
                    conv_block(rb)
                gate_rows(80, 96)
                for rb in range(19, 23):
                    conv_block(rb)
                gate_rows(96, 112)
                for rb in range(23, 27):
                    conv_block(rb)
                gate_rows(112, 128)
                for rb in range(27, 32):
                    conv_block(rb)

    nc.compile()
    return nc


def prep_inputs(x, w1, b1, g1, be1, m1, v1, wh, bh, ww, bw, wc, bc, g2, be2, m2, v2):
    """Host-side prep: per-core input maps (weights replicated)."""
    bf = ml_dtypes.bfloat16
    N = x.shape[0]
    s1 = (g1 / np.sqrt(v1 + EPS)).astype(np.float64)
    t1f = s1 * b1 + be1 - m1 * s1
    p8 = np.stack([s1 / 6.0, t1f / 6.0, s1, t1f + 3.0], axis=1).astype(np.float32)
    s2 = (g2 / np.sqrt(v2 + EPS)).astype(np.float64)
    b2 = bc * s2 + be2 - m2 * s2
    p128 = np.stack([bh, bw, s2, b2], axis=1).astype(np.float32)
    cc = np.array([7.0 / 128, 3.0 / 128, 1.0 / 128])
    w1ts = np.stack([w1.T * c for c in cc], axis=1)              # (C, 3, MIP)
    shared = {
        "w1t": np.ascontiguousarray(w1.T).astype(bf),            # (C, MIP)
        "w1ts": np.ascontiguousarray(w1ts.reshape(C, 3 * MIP)).astype(bf),
        "wht": np.ascontiguousarray(wh.T).astype(bf),            # (MIP, C)
        "wwt": np.ascontiguousarray(ww.T).astype(bf),            # (MIP, C)
        "wct": np.ascontiguousarray(
            np.transpose(wc, (1, 2, 3, 0)).reshape(C, 9 * C)
        ).astype(bf),                                            # [i, (ky kx), o]
        "p8": p8,
        "p128": p128,
    }
    in_maps = []
    for n in range(N):
        m = dict(shared)
        m["x"] = np.ascontiguousarray(x[n].reshape(C, H * W)).astype(bf)
        in_maps.append(m)
    return in_maps


def run(inputs, trace=False):
    if "nc" not in _CACHE:
        _CACHE["nc"] = build_nc()
    nc = _CACHE["nc"]
    in_maps = prep_inputs(**inputs)
    res = run_bass_kernel_spmd(nc, in_maps, core_ids=list(range(8)), trace=trace)
    out = np.stack([np.asarray(res.results[i]["out"]) for i in range(8)], axis=0)
    return out.astype(np.float32), res


def kernel(**inputs) -> np.ndarray:
    out, _ = run(inputs, trace=False)
    return out
